# revision 1
# baseline (speedup 1.0000x reference)
"""Trainium2 Bass kernel: 4096x4096 fp32 image, 16x16 valid cross-correlation + bias.

Phase-deinterleaved matmul conv (G=4): the host splits X's columns into
G=4 phases per 32-row strip,
    D[g*32+rho, n] = X[i0+rho, 4n+g],
so one banded-stationary matmul pass covers 4 kernel columns at once:
    psum[d*17+mu, J] += sum_p Sq[p, d*17+mu] * D[p, J+q]
    Sq[g*32+rho, d*17+mu] = w[rho-mu, 4q+g-d]   (when in range)
QP=5 passes replace the 16 column-shift passes of the direct banded
scheme — a ~3x cut in PE-streamed columns per output row. Each tile's
5-pass accumulation is split 1+2+2 across three psum banks (accumulating
matmuls stream ~40% slower than resetting ones) and recombined on the
Act + DVE engines. Outputs are written as phase planes [68, 1024] per strip; the
host re-interleaves.

All storage is bf16 (fp32 PSUM accumulation), halving HBM traffic; rel
err vs the fp32 reference ~3e-3, far under the 2e-2 gate. Strip loads
are column-split across the two HWDGE queues (sync/SP + scalar/Act);
outputs go on the gpsimd SWDGE queue. Output rows are sharded across
the 8 cores (510 rows each; the final output row is computed on host);
weights and bias replicated.

Env (bench only): CONV_LOOP wraps the body in a hardware For_i loop.
"""
import os

import numpy as np

import concourse.mybir as mybir
import concourse.tile as tile
from concourse import bacc
from concourse.bass_utils import run_bass_kernel_spmd

H = 4096
W = 4096
KH = 16
KW = 16
OH = H - KH + 1  # 4081
OW = W - KW + 1  # 4081
NCORES = 8
RPC = 510  # output rows per core (row 4080 is computed on host)
N_T = 512  # psum tile free size (one fp32 PSUM bank)

G = 4  # column phases
R = 128 // G  # 32 strip rows
OUT_R = R - KH + 1  # 17 output rows per strip
QP = (G - 1 + KW - 1) // G + 1  # 5 matmul passes per strip
M = G * OUT_R  # 68 psum partitions
NW = W // G  # 1024 deinterleaved columns
NWP = NW + 8  # padded deint width
SPC = 30  # strips per core (30*17 = 510 exactly)
NTILES = NW // N_T  # 2
CSPLIT = 516  # j=0 matmuls read cols 0..515; j=1 reads 512..1027

DT = mybir.dt.bfloat16
NP_BF16 = mybir.dt.np(mybir.dt.bfloat16)

_build_cache = {}


def _build():
    loop = int(os.environ.get("CONV_LOOP", "1"))
    if loop in _build_cache:
        return _build_cache[loop]

    nc = bacc.Bacc()
    x_d = nc.dram_tensor("Xd", [SPC, 128, NWP], DT, kind="ExternalInput")
    wq_d = nc.dram_tensor("wq", [128, QP, M], DT, kind="ExternalInput")
    bias_d = nc.dram_tensor("biasb", [128, 1], mybir.dt.float32, kind="ExternalInput")
    out_d = nc.dram_tensor("outp", [SPC, M, NW], DT, kind="ExternalOutput")

    with tile.TileContext(nc) as tc:
        with (
            tc.tile_pool(name="const", bufs=1) as cpool,
            tc.tile_pool(name="strip", bufs=4) as spool,
            tc.tile_pool(name="obuf", bufs=4) as opool,
            tc.tile_pool(name="tmp", bufs=4) as tpool,
            tc.tile_pool(name="psum", bufs=8, space="PSUM") as ppool,
        ):
            wq = cpool.tile([128, QP, M], DT)
            nc.gpsimd.dma_start(wq[:], wq_d[:])
            bias_sb = cpool.tile([128, 1], mybir.dt.float32)
            nc.gpsimd.dma_start(bias_sb[:], bias_d[:])

            def body():
                for s in range(SPC):
                    strip = spool.tile([128, NWP], DT, tag="strip")
                    nc.sync.dma_start(strip[:, 0:CSPLIT], x_d[s, :, 0:CSPLIT])
                    nc.scalar.dma_start(
                        strip[:, CSPLIT:NWP], x_d[s, :, CSPLIT:NWP]
                    )
                    ot = opool.tile([M, NW], DT, tag="ot")
                    for j in range(NTILES):
                        n0 = j * N_T
                        # Split the 5-pass chain 1+2+2 across three psum
                        # tiles: accumulating (start=False) matmuls stream
                        # ~300ns vs ~208ns for resetting ones, so more
                        # shorter chains are faster. Chains are emitted
                        # shortest-first (C, B, A) so each drain overlaps
                        # the later chains' matmuls. Drains: Act copies C
                        # (plain Copy is fast; the bias-AP Identity path is
                        # not), DVE combines with B (+bias) then A — each
                        # op reads at most one PSUM operand (walrus limit).
                        psc = ppool.tile(
                            [M, N_T], mybir.dt.float32, tag="ps",
                            name=f"psc{s}_{j}",
                        )
                        psb = ppool.tile(
                            [M, N_T], mybir.dt.float32, tag="ps",
                            name=f"psb{s}_{j}",
                        )
                        psa = ppool.tile(
                            [M, N_T], mybir.dt.float32, tag="ps",
                            name=f"psa{s}_{j}",
                        )
                        for pst, qs in (
                            (psc, (4,)),
                            (psb, (2, 3)),
                            (psa, (0, 1)),
                        ):
                            for qi, q in enumerate(qs):
                                nc.tensor.matmul(
                                    pst[:M, :N_T],
                                    wq[:, q, :M],
                                    strip[:, n0 + q : n0 + q + N_T],
                                    start=(qi == 0),
                                    stop=(qi == len(qs) - 1),
                                )
                        tc_sb = tpool.tile(
                            [M, N_T], mybir.dt.float32, tag="tc",
                            name=f"tc{s}_{j}",
                        )
                        t1_sb = tpool.tile(
                            [M, N_T], mybir.dt.float32, tag="t1",
                            name=f"t1{s}_{j}",
                        )
                        nc.scalar.copy(tc_sb[:M, :N_T], psc[:M, :N_T])
                        nc.vector.scalar_tensor_tensor(
                            t1_sb[:M, :N_T],
                            psb[:M, :N_T],
                            bias_sb[:M],
                            tc_sb[:M, :N_T],
                            mybir.AluOpType.add,
                            mybir.AluOpType.add,
                        )
                        nc.vector.scalar_tensor_tensor(
                            ot[:M, n0 : n0 + N_T],
                            psa[:M, :N_T],
                            0.0,
                            t1_sb[:M, :N_T],
                            mybir.AluOpType.add,
                            mybir.AluOpType.add,
                        )
                    nc.gpsimd.dma_start(out_d[s], ot[:M, :NW])

            if loop > 1:
                with tc.For_i(0, loop, 1):
                    body()
            else:
                body()
    nc.finalize()
    _build_cache[loop] = nc
    return nc


def _host_prep(X, weight, bias):
    Xb = np.ascontiguousarray(X.astype(NP_BF16))
    # last touched input row is RPC*7 + OUT_R*29 + 31 = 4094 < H: no padding
    # Xdint[g][row, n] = X[row, G*n+g]
    Xdint = np.ascontiguousarray(Xb.reshape(H, NW, G).transpose(2, 0, 1))

    wb = weight.astype(NP_BF16)
    wq = np.zeros((128, QP, M), dtype=NP_BF16)
    for q in range(QP):
        for g in range(G):
            for d in range(G):
                c = G * q + g - d
                if not (0 <= c < KW):
                    continue
                for mu in range(OUT_R):
                    wq[g * R + mu : g * R + mu + KH, q, d * OUT_R + mu] = wb[:, c]
    biasb = np.full((128, 1), np.float32(bias[0]), dtype=np.float32)

    starts = (
        np.arange(NCORES)[:, None] * RPC + np.arange(SPC)[None, :] * OUT_R
    )  # [NCORES, SPC]
    rows = starts[:, :, None] + np.arange(R)[None, None, :]  # [NCORES, SPC, R]
    gathered = Xdint[:, rows, :]  # [G, NCORES, SPC, R, NW]

    in_maps = []
    for c in range(NCORES):
        Xd = np.zeros((SPC, 128, NWP), dtype=NP_BF16)
        Xd[:, :, :NW] = gathered[:, c].transpose(1, 0, 2, 3).reshape(SPC, 128, NW)
        in_maps.append({"Xd": Xd, "wq": wq, "biasb": biasb})
    return in_maps


def _host_post(results):
    rows = []
    for c in range(NCORES):
        outp = np.asarray(results[c]["outp"])  # [SPC, M, NW] bf16
        blk = (
            outp.reshape(SPC, G, OUT_R, NW)
            .transpose(0, 2, 3, 1)
            .reshape(SPC * OUT_R, W)
        )
        rows.append(blk[:RPC])
    full = np.concatenate(rows, axis=0)  # [4080, W]
    return np.ascontiguousarray(full[:, :OW]).astype(np.float32)


def _final_row(X, weight, bias):
    """Output row OH-1 (the one row the 30-strip grid doesn't cover),
    computed exactly in fp32 on the host."""
    win = np.lib.stride_tricks.sliding_window_view(
        X[OH - 1 : OH - 1 + KH, :], (KH, KW)
    )[0]  # [OW, KH, KW]
    return np.einsum("aij,ij->a", win, weight, optimize=True) + bias[0]


def kernel(X, weight, bias):
    X = np.asarray(X, dtype=np.float32)
    weight = np.asarray(weight, dtype=np.float32)
    bias = np.asarray(bias, dtype=np.float32)
    nc = _build()
    in_maps = _host_prep(X, weight, bias)
    res = run_bass_kernel_spmd(nc, in_maps, core_ids=list(range(NCORES)))
    out = _host_post(res.results)
    last = _final_row(X, weight, bias).astype(np.float32)
    return np.vstack([out, last[None, :]])


def _run(X, weight, bias, dt_name=None, trace=False):
    """Compatibility entry for test.py: returns (output, results)."""
    X = np.asarray(X, dtype=np.float32)
    weight = np.asarray(weight, dtype=np.float32)
    bias = np.asarray(bias, dtype=np.float32)
    nc = _build()
    in_maps = _host_prep(X, weight, bias)
    res = run_bass_kernel_spmd(
        nc, in_maps, core_ids=list(range(NCORES)), trace=trace
    )
    out = _host_post(res.results)
    last = _final_row(X, weight, bias).astype(np.float32)
    return np.vstack([out, last[None, :]]), res



# revision 2
# speedup vs baseline: 2.1234x; 2.1234x over previous
"""Trainium2 Bass kernel v2: 4096x4096 fp32 image, 16x16 valid cross-corr + bias.

Block-output scheme: each psum element holds one output of a 16x8 block,
  psum[m=(a,b), n] = out[16s + a, 8n + b],
and the contraction dim holds a 16x8 patch of X,
  D_t[p=(rho,gamma), n] = X[16t + rho, 8n + gamma]
(i.e. X reshaped so partition = (row%16)*8 + col%8, free = col//8).
Each kernel tap (r, c) decomposes uniquely as r = rho - a + 16u (u in {0,1}),
c = gamma - b + 8v (v in {0,1,2}), so SIX matmuls accumulate the full 16x16
conv for 16 output rows x 4096 cols at once:
  psum += S_uv.T @ D_{s+u}[:, v:v+512],
  S_uv[p, m] = W[rho - a + 16u, gamma - b + 8v] (where in range, else 0).
This streams 6x512 PE columns per 16 output rows (vs 10x512 per 17 rows
for the banded-phase scheme) and loads each X byte exactly once (strips
stride = strip height; no halo duplication in HBM traffic).

Per strip: one 6-matmul chain into a single PSUM bank, then one DVE
tensor_scalar_add (+bias, bf16 cast) to SBUF, then SWDGE store. All
storage bf16 (fp32 PSUM accumulation); rel err vs fp32 reference ~3e-3.
Output rows sharded across 8 cores (512 rows each, 32 strips of 16);
weights and bias replicated.

Env (bench only): CONV_LOOP wraps the body in a hardware For_i loop.
"""
import os

import numpy as np

import concourse.mybir as mybir
import concourse.tile as tile
from concourse import bacc
from concourse.bass_utils import run_bass_kernel_spmd

H = 4096
W = 4096
KH = 16
KW = 16
OH = H - KH + 1  # 4081
OW = W - KW + 1  # 4081
NCORES = 8

A = 16  # output block rows (= strip height)
B = 8  # output block cols
NB = W // B  # 512 bases per strip row
TW = NB + 4  # tile width, padded for the v-shift (n+v <= 513)
SPT = 32  # strips per core (32*16 = 512 output rows/core)
NS = 256  # total strips (covers output rows 0..4095)
NU, NV = 2, 3  # row / col pass counts

DT = mybir.dt.bfloat16
NP_BF16 = mybir.dt.np(mybir.dt.bfloat16)

_build_cache = {}


def _build():
    loop = int(os.environ.get("CONV_LOOP", "1"))
    if loop in _build_cache:
        return _build_cache[loop]

    nc = bacc.Bacc()
    xt_d = nc.dram_tensor("xt", [SPT + 1, 128, TW], DT, kind="ExternalInput")
    wq_d = nc.dram_tensor("wq", [128, NU * NV, 128], DT, kind="ExternalInput")
    bias_d = nc.dram_tensor("biasb", [128, 1], mybir.dt.float32, kind="ExternalInput")
    out_d = nc.dram_tensor("outp", [SPT, 128, NB], DT, kind="ExternalOutput")

    with tile.TileContext(nc) as tc:
        with (
            tc.tile_pool(name="const", bufs=1) as cpool,
            tc.tile_pool(name="xtiles", bufs=SPT + 2) as spool,
            tc.tile_pool(name="obuf", bufs=4) as opool,
            tc.tile_pool(name="psum", bufs=4, space="PSUM") as ppool,
        ):
            wq = cpool.tile([128, NU * NV, 128], DT)
            nc.scalar.dma_start(wq[:], wq_d[:])
            bias_sb = cpool.tile([128, 1], mybir.dt.float32)
            nc.scalar.dma_start(bias_sb[:], bias_d[:])

            def body():
                tiles = {}

                def get_tile(t):
                    if t not in tiles:
                        tt = spool.tile([128, TW], DT, tag="xt", name=f"xt{t}")
                        nc.sync.dma_start(tt[:], xt_d[t])
                        tiles[t] = tt
                    return tiles[t]

                for s in range(SPT):
                    t0 = get_tile(s)
                    t1 = get_tile(s + 1)
                    ps = ppool.tile(
                        [128, NB], mybir.dt.float32, tag="ps", name=f"ps{s}"
                    )
                    k = 0
                    for tt in (t0, t1):
                        for v in range(NV):
                            nc.tensor.matmul(
                                ps[:, :],
                                wq[:, k, :],
                                tt[:, v : v + NB],
                                start=(k == 0),
                                stop=(k == NU * NV - 1),
                            )
                            k += 1
                    ot = opool.tile([128, NB], DT, tag="ot", name=f"ot{s}")
                    nc.vector.tensor_scalar_add(ot[:, :], ps[:, :], bias_sb[:])
                    nc.gpsimd.dma_start(out_d[s], ot[:, :])

            if loop > 1:
                with tc.For_i(0, loop, 1):
                    body()
            else:
                body()
    nc.finalize()
    _build_cache[loop] = nc
    return nc


def _host_prep(X, weight, bias):
    Xb = np.ascontiguousarray(X.astype(NP_BF16))
    # Pad rows to (NS+1)*A and cols to TW*B with zeros, then reshape so
    # Xr[t, rho*8+gamma, n] = Xpad[A*t + rho, B*n + gamma].
    Xp = np.zeros(((NS + 1) * A, TW * B), dtype=NP_BF16)
    Xp[:H, :W] = Xb
    Xr = np.ascontiguousarray(
        Xp.reshape(NS + 1, A, TW, B).transpose(0, 1, 3, 2).reshape(NS + 1, 128, TW)
    )

    wb = weight.astype(NP_BF16)
    # wq[rho*8+gamma, u*NV+v, a*8+b] = W[rho-a+16u, gamma-b+8v] where valid
    wq = np.zeros((128, NU * NV, 128), dtype=NP_BF16)
    rho = np.arange(A)[:, None, None, None]  # [A,1,1,1]
    gam = np.arange(B)[None, :, None, None]  # [1,B,1,1]
    aa = np.arange(A)[None, None, :, None]  # [1,1,A,1]
    bb = np.arange(B)[None, None, None, :]  # [1,1,1,B]
    for u in range(NU):
        for v in range(NV):
            r = rho - aa + 16 * u  # [A,1,A,1]
            c = gam - bb + 8 * v  # [1,B,1,B]
            valid = (0 <= r) & (r < KH) & (0 <= c) & (c < KW)
            vals = wb[np.clip(r, 0, KH - 1), np.clip(c, 0, KW - 1)]
            vals = np.where(valid, vals, np.zeros((), dtype=NP_BF16))
            wq[:, u * NV + v, :] = vals.reshape(128, 128)
    biasb = np.full((128, 1), np.float32(bias[0]), dtype=np.float32)

    in_maps = []
    for c in range(NCORES):
        xt = np.ascontiguousarray(Xr[SPT * c : SPT * c + SPT + 1])
        in_maps.append({"xt": xt, "wq": wq, "biasb": biasb})
    return in_maps


def _host_post(results):
    rows = []
    for c in range(NCORES):
        outp = np.asarray(results[c]["outp"])  # [SPT, 128, NB] bf16
        blk = (
            outp.reshape(SPT, A, B, NB)
            .transpose(0, 1, 3, 2)
            .reshape(SPT * A, W)
        )
        rows.append(blk)
    full = np.concatenate(rows, axis=0)  # [4096, 4096]
    return np.ascontiguousarray(full[:OH, :OW]).astype(np.float32)


def kernel(X, weight, bias):
    X = np.asarray(X, dtype=np.float32)
    weight = np.asarray(weight, dtype=np.float32)
    bias = np.asarray(bias, dtype=np.float32)
    nc = _build()
    in_maps = _host_prep(X, weight, bias)
    res = run_bass_kernel_spmd(nc, in_maps, core_ids=list(range(NCORES)))
    return _host_post(res.results)


def _run(X, weight, bias, dt_name=None, trace=False):
    """Compatibility entry for test.py: returns (output, results)."""
    X = np.asarray(X, dtype=np.float32)
    weight = np.asarray(weight, dtype=np.float32)
    bias = np.asarray(bias, dtype=np.float32)
    nc = _build()
    in_maps = _host_prep(X, weight, bias)
    res = run_bass_kernel_spmd(
        nc, in_maps, core_ids=list(range(NCORES)), trace=trace
    )
    return _host_post(res.results), res
